# revision 86
# baseline (speedup 1.0000x reference)
"""GQA attention layer (dense_transformer) on 8 Trainium2 NeuronCores.

Sharding: data-parallel over batch (2) x tensor-parallel over head groups (4).
Core c handles batch c//4 and head-group c%4 (8 q heads, 2 kv heads).
Each core computes a partial output (its heads' contribution through its
Wo row-slice); the host sums the 4 partials per batch.

Per-core pipeline (all matmuls bf16, fp32 accumulation), fully software-
pipelined so the PE (tensor) engine never waits on ScalarE exp:

  P1  (fused QKV projection): per token tile it, one hst load feeds
      q0/q1/kv projections (96 matmuls); RMSNorm + RoPE (tables carry the
      128^-0.25 score scale and the norm weight) on Vector/Scalar engines;
      per-group batched transposes (1-iteration lag) -> qT/kT [d, i];
      v stored token-major with a ones column for softmax denominators.
  P2  (attention) woven with P3 (output projection of the previous block):
      scoresT[j,i] = kT.T @ qT per jt-pair into a [128,1024] PSUM tile,
      exp on ScalarE (tail-sliced causally), triangular mask only on the
      exact-diagonal 128x128 tiles, PV with v_aug accumulates attn_out and
      row sums; normalize by reciprocal during eviction; batched transpose
      -> aoT. P3 chunks ([128,1024] of out = aoT.T @ Wo) are emitted
      between score pairs so exp latency hides under P3 matmuls.
"""
import math
import os
import sys
from collections import deque
from contextlib import ExitStack

import numpy as np

_REPO = "/opt/trn_rl_repo"
_PKGS = "/opt/pypackages"
for _p in (_REPO, _PKGS):
    if _p not in sys.path:
        sys.path.append(_p)

import ml_dtypes

BF16 = ml_dtypes.bfloat16

B, S, HIDDEN = 2, 2048, 4096
NUM_HEADS, NUM_KV_HEADS, HEAD_DIM = 32, 8, 128
EPS = 1e-6
ROPE_THETA = 10000.0
N_CORES = 8
TP = 4  # head groups
HQ = NUM_HEADS // TP        # 8 q heads per core
HKV = NUM_KV_HEADS // TP    # 2 kv heads per core
KT = HIDDEN // 128          # 32 k tiles
IT = S // 128               # 16 token tiles
IB = S // 512               # 4 token blocks (512 wide)


def _split_drain_waits():
    """walrus here rejects >1 sync wait on the tail Drain; split them."""
    from concourse import mybir
    from concourse.tile import TileContext
    from concourse.vector_clock import ScopedClock

    def _drain_and_barrier(self, tick_clock, wait_clock):
        drain_inst = self.nc.sync.drain()
        wait_clock.add_sem_waits(
            drain_inst.ins, ScopedClock({None: tick_clock.global_clock})
        )
        inst = drain_inst.ins
        si = inst.sync_info
        if si is not None and si.on_wait is not None and len(si.on_wait) > 1:
            waits = list(si.on_wait)
            del si.on_wait[1:]
            for i in range(1, len(waits)):
                e_inst = self.nc.sync.drain().ins
                if e_inst.sync_info is None:
                    e_inst.sync_info = mybir.SyncInfo(on_wait=[], on_update=[])
                e_inst.sync_info.on_wait.extend(waits[i : i + 1])
        self.nc.all_engine_barrier()
        assert self.sems is not None
        popped = self.nc._tile_sem_poison_stack.pop()
        assert popped is self._sem_poison
        self.nc.clear_and_free_semaphores(list(self.sems.allocated().values()))
        self.nc.all_engine_barrier()
        _fixup_wait_limits(self.nc)

    TileContext._drain_and_barrier = _drain_and_barrier


def _fixup_wait_limits(nc):
    """walrus in this image caps sync waits per instruction (DMA: hit at 3,
    Drain at 4+). Hoist excess waits onto nop instructions inserted just
    before the offender on the same engine (waits still complete before the
    original program point; engine order preserves semantics)."""
    from concourse import mybir

    def limit_for(inst):
        return 1

    def mk_nop(engine):
        bi = nc.engines[engine].nop(nofuse=True)
        inst = bi.ins if hasattr(bi, "ins") else bi
        for f in nc.m.functions:
            for blk in f.blocks:
                if blk.instructions and blk.instructions[-1] is inst:
                    blk.instructions.pop()
        return inst

    for f in nc.m.functions:
        for blk in f.blocks:
            out = []
            for inst in blk.instructions:
                si = inst.sync_info
                nw = len(si.on_wait) if si is not None and si.on_wait else 0
                lim = limit_for(inst)
                if nw > lim:
                    waits = list(si.on_wait)
                    del si.on_wait[lim:]
                    for w in waits[lim:]:
                        nop = mk_nop(inst.engine)
                        nop.sync_info = mybir.SyncInfo(on_wait=[w], on_update=[])
                        out.append(nop)
                out.append(inst)
            blk.instructions[:] = out


def build_bass():
    import concourse.bass as bass
    import concourse.tile as tile
    from concourse import mybir

    _split_drain_waits()

    f32 = mybir.dt.float32
    bf16 = mybir.dt.bfloat16
    AF = mybir.ActivationFunctionType
    ALU = mybir.AluOpType

    nc = bass.Bass("TRN2", target_bir_lowering=False, debug=False)

    hst = nc.dram_tensor("hst", [IT, 128, KT, 128], bf16, kind="ExternalInput")
    wq = nc.dram_tensor("wq", [128, KT, HQ * 128], bf16, kind="ExternalInput")
    wkv = nc.dram_tensor("wkv", [128, KT, 4 * 128], bf16, kind="ExternalInput")
    wo = nc.dram_tensor("wo", [128, HQ, HIDDEN], bf16, kind="ExternalInput")
    cosq = nc.dram_tensor("cosq", [128, IT, 128], bf16, kind="ExternalInput")
    sinq = nc.dram_tensor("sinq", [128, IT, 128], bf16, kind="ExternalInput")
    cosk = nc.dram_tensor("cosk", [128, IT, 128], bf16, kind="ExternalInput")
    sink = nc.dram_tensor("sink", [128, IT, 128], bf16, kind="ExternalInput")
    trim = nc.dram_tensor("trim", [128, 128], bf16, kind="ExternalInput")
    ident = nc.dram_tensor("ident", [128, 128], bf16, kind="ExternalInput")
    out = nc.dram_tensor("out", [S, HIDDEN], f32, kind="ExternalOutput")

    with tile.TileContext(nc) as tc, ExitStack() as top:
        const = top.enter_context(tc.tile_pool(name="const", bufs=1))
        res = top.enter_context(tc.tile_pool(name="res", bufs=1))

        # Persistent results shared across phases (aoT lives in the P2+P3
        # scope to keep P1's SBUF footprint down)
        qT = res.tile([128, HQ, S], bf16, tag="qT")
        kT = res.tile([128, HKV, S], bf16, tag="kT")
        v_aug = res.tile([128, HKV, IT, 129], bf16, tag="vaug")
        # rq tiles outlive P1's pools: the last token tile's transposes are
        # deferred into the attention weave.
        rqpool = top.enter_context(tc.tile_pool(name="rqp", bufs=6))

        ident_sb = const.tile([128, 128], bf16, tag="ident")
        tri_sb = const.tile([128, 128], bf16, tag="trim")
        eps_sb = const.tile([128, 1], f32, tag="eps")
        nc.vector.memset(eps_sb, EPS)
        nc.vector.memset(v_aug[:, :, :, 128:129], 1.0)

        # ---------------- Phase 1: KV then Q projection ----------------
        # KV first: it only needs Wk/Wv (4.2MB), so the PE starts ~5us in
        # while Wq (8.4MB) streams during the whole KV pass.
        NWC = 8  # weight chunks (4 kt each)
        with ExitStack() as p1:
            wpool = p1.enter_context(tc.tile_pool(name="wp", bufs=1))
            hpool = p1.enter_context(tc.tile_pool(name="hst", bufs=2))
            qpp = p1.enter_context(tc.tile_pool(name="qpp", bufs=2, space="PSUM"))
            qtps = p1.enter_context(tc.tile_pool(name="qtp", bufs=1, space="PSUM"))
            stage = p1.enter_context(tc.tile_pool(name="stg", bufs=2))
            small = p1.enter_context(tc.tile_pool(name="sml", bufs=4))
            # KV-pass pools on the RIGHT of SBUF/PSUM: when they close, the
            # early-attention pools take that space, so the first exp/PV of
            # the attention phase don't inherit the full P1 pool barrier.
            kvstack = ExitStack()
            wkvpool = kvstack.enter_context(
                tc.tile_pool(name="wkvp", bufs=1, side="right"))
            kvpp = kvstack.enter_context(
                tc.tile_pool(name="kvp", bufs=2, space="PSUM", side="right"))
            ktps = kvstack.enter_context(
                tc.tile_pool(name="ktp", bufs=1, space="PSUM", side="right"))

            ht0 = hpool.tile([128, KT, 128], bf16, tag="ht", name="ht_0")
            nc.sync.dma_start(out=ht0, in_=hst.ap()[0])
            wq_c = []
            wkv_c = []
            for c in range(NWC):
                wqc = wpool.tile([128, 4, HQ * 128], bf16, tag=f"wq{c}",
                                 name=f"wq_c{c}")
                wkvc = wkvpool.tile([128, 4, 512], bf16, tag=f"wkv{c}",
                                    name=f"wkv_c{c}")
                wq_c.append(wqc)
                wkv_c.append(wkvc)
            cos_sb = {}
            sin_sb = {}
            cos_sb["q"] = wpool.tile([128, IT, 128], bf16, tag="cosq", name="cosq_sb")
            sin_sb["q"] = wpool.tile([128, IT, 128], bf16, tag="sinq", name="sinq_sb")
            cos_sb["k"] = wkvpool.tile([128, IT, 128], bf16, tag="cosk",
                                       name="cosk_sb")
            sin_sb["k"] = wkvpool.tile([128, IT, 128], bf16, tag="sink",
                                       name="sink_sb")
            for c in range(NWC):
                nc.sync.dma_start(out=wkv_c[c],
                                  in_=wkv.ap()[:, c * 4 : (c + 1) * 4, :])
                if c == 1:
                    nc.sync.dma_start(out=cos_sb["k"], in_=cosk.ap())
                    nc.sync.dma_start(out=sin_sb["k"], in_=sink.ap())
                    nc.sync.dma_start(out=ident_sb, in_=ident.ap())
            nc.sync.dma_start(out=tri_sb, in_=trim.ap())
            # Wq chunks + q tables are issued inside the KV loop so the
            # KV pass's hst prefetches aren't starved behind them.

            def rms_norm(psum_t, n_heads, which, it, stage, small):
                """psum_t: [128 i, n_heads*128] raw projections (PSUM).
                RMS-normalize each head into a staged qn tile; PSUM is free
                once the qn muls retire."""
                w = n_heads * 128
                sq = stage.tile([128, 512], f32, tag="sq",
                                name=f"sq_{which}_{it}")
                nc.scalar.activation(out=sq[:, 0:w], in_=psum_t, func=AF.Square)
                ss = small.tile([128, 4], f32, tag="ssr", name=f"ss_{which}_{it}")
                nc.vector.tensor_reduce(
                    out=ss[:, 0:n_heads],
                    in_=sq[:, 0:w].rearrange("p (h d) -> p h d", h=n_heads),
                    op=ALU.add, axis=mybir.AxisListType.X,
                )
                # rstd = exp(-0.5*ln(ms+eps)): keeps the whole kernel on the
                # natural_log_exp activation table (no table reloads, and the
                # attention exps can interleave freely)
                rstd = small.tile([128, 4], f32, tag="rstd",
                                  name=f"rstd_{which}_{it}")
                nc.scalar.activation(
                    out=rstd[:, 0:n_heads], in_=ss[:, 0:n_heads], func=AF.Ln,
                    scale=1.0 / HEAD_DIM, bias=eps_sb,
                )
                nc.scalar.activation(
                    out=rstd[:, 0:n_heads], in_=rstd[:, 0:n_heads], func=AF.Exp,
                    scale=-0.5,
                )
                qn = stage.tile([128, 512], f32, tag="qn", bufs=4,
                                name=f"qn_{which}_{it}")
                for h in range(n_heads):
                    nc.vector.tensor_scalar_mul(
                        out=qn[:, h * 128 : (h + 1) * 128],
                        in0=psum_t[:, h * 128 : (h + 1) * 128],
                        scalar1=rstd[:, h : h + 1],
                    )
                return qn

            def rope(qn, n_heads, which, it, stage):
                w = n_heads * 128
                qn3 = qn[:, 0:w].rearrange("p (h d) -> p h d", h=n_heads)
                cos_t = cos_sb["q" if which.startswith("q") else "k"][:, it, :]
                sin_t = sin_sb["q" if which.startswith("q") else "k"][:, it, :]
                ct = cos_t[:, 0:64][:, None, :].broadcast_to([128, n_heads, 64])
                cb = cos_t[:, 64:128][:, None, :].broadcast_to([128, n_heads, 64])
                st_ = sin_t[:, 0:64][:, None, :].broadcast_to([128, n_heads, 64])
                sb_ = sin_t[:, 64:128][:, None, :].broadcast_to([128, n_heads, 64])
                ta = stage.tile([128, 4, 64], f32, tag="ta",
                                name=f"ta_{which}_{it}")
                tb = stage.tile([128, 4, 64], f32, tag="tb",
                                name=f"tb_{which}_{it}")
                rq = rqpool.tile([128, 512], bf16, tag="rq",
                                 name=f"rq_{which}_{it}")
                rq3 = rq[:, 0:w].rearrange("p (h d) -> p h d", h=n_heads)
                nc.vector.tensor_mul(out=ta[:, 0:n_heads], in0=qn3[:, :, 0:64], in1=ct)
                nc.vector.tensor_mul(out=tb[:, 0:n_heads], in0=qn3[:, :, 64:128], in1=st_)
                nc.vector.tensor_sub(out=rq3[:, :, 0:64], in0=ta[:, 0:n_heads],
                                     in1=tb[:, 0:n_heads])
                nc.vector.tensor_mul(out=ta[:, 0:n_heads], in0=qn3[:, :, 64:128], in1=cb)
                nc.vector.tensor_mul(out=tb[:, 0:n_heads], in0=qn3[:, :, 0:64], in1=sb_)
                nc.vector.tensor_add(out=rq3[:, :, 64:128], in0=ta[:, 0:n_heads],
                                     in1=tb[:, 0:n_heads])
                return rq

            pend_k = []   # (rk, it) awaiting transposition
            pend = []     # (rq_q0, rq_q1, it) awaiting transposition
            qn_pend = []  # (qn0, qn1, it) awaiting rope

            def emit_q_transposes(rq_q0, rq_q1, it):
                # q: two groups of 4 heads, each batched into one PSUM tile
                for gi, rqg in enumerate((rq_q0, rq_q1)):
                    pt_q = qtps.tile([128, 512], bf16, tag="qtr",
                                     name=f"qtr_{it}_{gi}")
                    for h in range(4):
                        nc.tensor.transpose(
                            pt_q[:, h * 128 : (h + 1) * 128],
                            rqg[:, h * 128 : (h + 1) * 128], ident_sb)
                    nc.scalar.activation(
                        out=qT[:, gi * 4 : gi * 4 + 4, it * 128 : (it + 1) * 128],
                        in_=pt_q.rearrange("p (h x) -> p h x", h=4),
                        func=AF.Copy)

            def emit_k_transposes(rk, it, pool=None, tag="ktr"):
                pool = pool if pool is not None else ktps
                pt_k = pool.tile([128, 256] if pool is ktps else [128, 512],
                                 bf16, tag=tag, name=f"ktr_{it}")
                for g in range(HKV):
                    nc.tensor.transpose(
                        pt_k[:, g * 128 : (g + 1) * 128],
                        rk[:, g * 128 : (g + 1) * 128], ident_sb)
                nc.vector.tensor_copy(
                    out=kT[:, :, it * 128 : (it + 1) * 128],
                    in_=pt_k[:, 0:256].rearrange("p (h x) -> p h x", h=HKV))

            # PE warmup: dummy matmuls on a zeroed tile while the first
            # weight/activation DMAs land, so real work starts at full clock
            warm = stage.tile([128, 512], bf16, tag="warm", name="warm")
            nc.vector.memset(warm, 0.0)
            wps = kvpp.tile([128, 512], f32, tag="pskv", name="warm_ps")
            for _ in range(15):
                nc.tensor.matmul(wps[:], warm[:, 0:128], warm[:, 0:512],
                                 start=True, stop=True)

            # -------- KV pass --------
            ht_next = [None]

            def prefetch_ht(it, nm):
                t = hpool.tile([128, KT, 128], bf16, tag="ht", name=f"{nm}_{it}")
                nc.sync.dma_start(out=t, in_=hst.ap()[it])
                return t

            for it in range(IT):
                ht = ht0 if it == 0 else ht_next[0]
                ht_next[0] = (prefetch_ht(it + 1, "htk") if it + 1 < IT
                              else prefetch_ht(0, "htq"))
                if it % 2 == 1:
                    c = it // 2
                    nc.sync.dma_start(
                        out=wq_c[c], in_=wq.ap()[:, c * 4 : (c + 1) * 4, :])
                if it == 8:
                    nc.sync.dma_start(out=cos_sb["q"], in_=cosq.ap())
                    nc.sync.dma_start(out=sin_sb["q"], in_=sinq.ap())
                pskv = kvpp.tile([128, 512], f32, tag="pskv", name=f"pskv_{it}")
                for kt in range(KT):
                    c, kl = divmod(kt, 4)
                    nc.tensor.matmul(pskv[:], ht[:, kt, :], wkv_c[c][:, kl, :],
                                     start=(kt == 0), stop=(kt == KT - 1))
                    if it == 0 and kt % 4 == 3 and kt < 28:
                        # first iteration races the wkv chunk DMAs: pad each
                        # chunk boundary with dummies so the inter-chunk
                        # stalls don't reset the PE p-state ramp
                        for _ in range(3):
                            nc.tensor.matmul(wps[:], warm[:, 0:128],
                                             warm[:, 0:512],
                                             start=True, stop=True)
                qnk = rms_norm(pskv[:, 0:256], 2, "k", it, stage, small)
                rk = rope(qnk, 2, "k", it, stage)
                for g in range(HKV):
                    sl = pskv[:, 256 + g * 128 : 256 + g * 128 + 128]
                    nc.scalar.activation(out=v_aug[:, g, it, 0:128], in_=sl,
                                         func=AF.Copy)
                pend_k.append((rk, it))
                if len(pend_k) > 1:
                    emit_k_transposes(*pend_k.pop(0))

            # KV pools retire; early-attention pools take their (right-side)
            # space so their tiles carry no dependency on the Q pass.
            kvstack.close()
            ptpool = top.enter_context(
                tc.tile_pool(name="ptp", bufs=2, side="right"))
            aopool = top.enter_context(
                tc.tile_pool(name="aop", bufs=2, side="right"))
            small2 = top.enter_context(
                tc.tile_pool(name="sm2", bufs=4, side="right"))
            opsum = top.enter_context(
                tc.tile_pool(name="ops", bufs=2, space="PSUM", side="right"))
            atps = top.enter_context(
                tc.tile_pool(name="atp", bufs=1, space="PSUM", side="right"))

            # -------- Q pass --------
            for it in range(IT):
                ht = ht_next[0]
                if it + 1 < IT:
                    ht_next[0] = prefetch_ht(it + 1, "htq")
                psq0 = qpp.tile([128, 512], f32, tag="psq0", name=f"psq0_{it}")
                psq1 = qpp.tile([128, 512], f32, tag="psq1", name=f"psq1_{it}")
                for kt in range(KT):
                    c, kl = divmod(kt, 4)
                    st = kt == 0
                    sp = kt == KT - 1
                    nc.tensor.matmul(psq0[:], ht[:, kt, :], wq_c[c][:, kl, 0:512],
                                     start=st, stop=sp)
                    nc.tensor.matmul(psq1[:], ht[:, kt, :], wq_c[c][:, kl, 512:1024],
                                     start=st, stop=sp)
                if it == 0:
                    # last KV-pass transposes, now that its RoPE chain
                    # drained (ktps is closed; use the attention psum pool)
                    rk_l, it_l = pend_k.pop(0)
                    emit_k_transposes(rk_l, it_l, pool=atps, tag="aot")
                # norm/rope pipelined one iteration apart: iteration it's qn
                # muls (which free the PSUM banks) are never queued behind a
                # previous iteration's RoPE ops on the DVE
                qn0 = rms_norm(psq0[:, :], 4, "q0", it, stage, small)
                qn1 = rms_norm(psq1[:, :], 4, "q1", it, stage, small)
                qn_pend.append((qn0, qn1, it))
                if len(qn_pend) > 1:
                    q0p, q1p, itp = qn_pend.pop(0)
                    pend.append((rope(q0p, 4, "q0", itp, stage),
                                 rope(q1p, 4, "q1", itp, stage), itp))
                    if len(pend) > 2:
                        emit_q_transposes(*pend.pop(0))
            q0p, q1p, itp = qn_pend.pop(0)
            pend.append((rope(q0p, 4, "q0", itp, stage),
                         rope(q1p, 4, "q1", itp, stage), itp))
            emit_q_transposes(*pend.pop(0))  # it=13 (rope long done)
            # it=14/15's q transposes are deferred into the attention weave
            # so the PE doesn't stall on their RoPE chains at the boundary.

        # ---------------- Phase 2+3: attention woven with out-proj ------
        with ExitStack() as p23:
            wopool = p23.enter_context(tc.tile_pool(name="wop", bufs=1))
            spsum = p23.enter_context(tc.tile_pool(name="sps", bufs=2, space="PSUM"))
            ostage = p23.enter_context(tc.tile_pool(name="ost", bufs=4))
            res23 = p23.enter_context(tc.tile_pool(name="rs2", bufs=1))
            aoT = res23.tile([128, HQ, S], bf16, tag="aoT")

            # Prefetch Wo now; transfers overlap the whole attention phase.
            wo_c = []
            for c in range(4):
                woc = wopool.tile([128, HQ, 1024], bf16, tag=f"wo{c}",
                                  name=f"wo_c{c}")
                wo_c.append(woc)
                nc.sync.dma_start(
                    out=woc, in_=wo.ap()[:, :, c * 1024 : (c + 1) * 1024])

            def emit_score_pair(h, ib, p):
                """Emit matmuls + exp for score pair p of head h, block ib.
                Returns {jt: (pt_tile, base_col, first_itl)}."""
                g = h // (HQ // HKV)
                qblk = qT[:, h, ib * 512 : (ib + 1) * 512]
                ps = spsum.tile([128, 1024], f32, tag="ss", name=f"ss_{ib}_{h}_{p}")
                pt = ptpool.tile([128, 1024], bf16, tag=f"pt{p}",
                                 name=f"pt_{ib}_{h}_{p}")
                ent = {}
                if p < 2 * ib:  # full pair: jt = 2p, 2p+1, all 512 i-cols
                    for half in range(2):
                        jt = 2 * p + half
                        nc.tensor.matmul(
                            ps[:, half * 512 : (half + 1) * 512],
                            kT[:, g, jt * 128 : (jt + 1) * 128], qblk,
                            start=True, stop=True)
                        ent[jt] = (pt, half * 512, 0)
                    nc.scalar.activation(out=pt, in_=ps, func=AF.Exp)
                elif p == 2 * ib:  # diag pair 1: jt=4ib (w 512), 4ib+1 (w 384)
                    jt0 = 4 * ib
                    nc.tensor.matmul(ps[:, 0:512],
                                     kT[:, g, jt0 * 128 : (jt0 + 1) * 128],
                                     qblk, start=True, stop=True)
                    nc.tensor.matmul(ps[:, 512:896],
                                     kT[:, g, (jt0 + 1) * 128 : (jt0 + 2) * 128],
                                     qblk[:, 128:512], start=True, stop=True)
                    nc.scalar.activation(out=pt[:, 0:896], in_=ps[:, 0:896],
                                         func=AF.Exp)
                    nc.vector.tensor_mul(out=pt[:, 0:128], in0=pt[:, 0:128],
                                         in1=tri_sb)
                    nc.vector.tensor_mul(out=pt[:, 512:640], in0=pt[:, 512:640],
                                         in1=tri_sb)
                    ent[jt0] = (pt, 0, 0)
                    ent[jt0 + 1] = (pt, 512, 1)
                else:  # diag pair 2: jt=4ib+2 (w 256), 4ib+3 (w 128)
                    jt0 = 4 * ib + 2
                    nc.tensor.matmul(ps[:, 0:256],
                                     kT[:, g, jt0 * 128 : (jt0 + 1) * 128],
                                     qblk[:, 256:512], start=True, stop=True)
                    nc.tensor.matmul(ps[:, 256:384],
                                     kT[:, g, (jt0 + 1) * 128 : (jt0 + 2) * 128],
                                     qblk[:, 384:512], start=True, stop=True)
                    nc.scalar.activation(out=pt[:, 0:384], in_=ps[:, 0:384],
                                         func=AF.Exp)
                    nc.vector.tensor_mul(out=pt[:, 0:128], in0=pt[:, 0:128],
                                         in1=tri_sb)
                    nc.vector.tensor_mul(out=pt[:, 256:384], in0=pt[:, 256:384],
                                         in1=tri_sb)
                    ent[jt0] = (pt, 0, 2)
                    ent[jt0 + 1] = (pt, 256, 3)
                return ent

            def emit_pv_unit(h, ib, itl, pts, ao_blk):
                """PV for token tile it = 4*ib+itl of head h + normalize."""
                g = h // (HQ // HKV)
                it_g = ib * 4 + itl
                po = opsum.tile([128, 129], f32, tag="po", name=f"po_{ib}_{h}_{itl}")
                for jt in range(it_g + 1):
                    pt, base, first = pts[jt]
                    sl = pt[:, base + (itl - first) * 128 : base + (itl - first) * 128 + 128]
                    nc.tensor.matmul(po[:], sl, v_aug[:, g, jt, :],
                                     start=(jt == 0), stop=(jt == it_g))
                rec = small2.tile([128, 1], f32, tag="rec", name=f"rec_{ib}_{h}_{itl}")
                nc.vector.reciprocal(out=rec, in_=po[:, 128:129])
                nc.vector.tensor_scalar_mul(
                    out=ao_blk[:, itl * 128 : (itl + 1) * 128],
                    in0=po[:, 0:128], scalar1=rec)

            def emit_ao_transpose_a(h, ib, ao_blk, cell):
                pt_a = atps.tile([128, 512], bf16, tag="aot", name=f"aot_{ib}_{h}")
                cell.append(pt_a)
                for itl in range(2):
                    nc.tensor.transpose(
                        pt_a[:, itl * 128 : (itl + 1) * 128],
                        ao_blk[:, itl * 128 : (itl + 1) * 128], ident_sb)

            def emit_ao_transpose_b(h, ib, ao_blk, cell):
                pt_a = cell[0]
                for itl in range(2, 4):
                    nc.tensor.transpose(
                        pt_a[:, itl * 128 : (itl + 1) * 128],
                        ao_blk[:, itl * 128 : (itl + 1) * 128], ident_sb)
                nc.vector.tensor_copy(
                    out=aoT[:, h, ib * 512 : (ib + 1) * 512], in_=pt_a)

            def emit_deferred_q(pi, gi):
                """it=14/15's q transposes, moved past the phase boundary."""
                rq0, rq1, it = pend[pi]
                rqg = (rq0, rq1)[gi]
                pt_q = atps.tile([128, 512], bf16, tag="aot",
                                 name=f"dqtr_{pi}_{gi}")
                for h in range(4):
                    nc.tensor.transpose(
                        pt_q[:, h * 128 : (h + 1) * 128],
                        rqg[:, h * 128 : (h + 1) * 128], ident_sb)
                nc.vector.tensor_copy(
                    out=qT[:, gi * 4 : gi * 4 + 4, it * 128 : (it + 1) * 128],
                    in_=pt_q.rearrange("p (h x) -> p h x", h=4))

            o_ec = [0]

            def emit_p3_chunk(m, c):
                """One [128,1024] chunk of out row-tile m: contract 8 heads."""
                ps = spsum.tile([128, 1024], f32, tag="ss", name=f"ps3_{m}_{c}")
                for k in range(HQ):
                    for i2 in range(2):
                        nc.tensor.matmul(
                            ps[:, i2 * 512 : (i2 + 1) * 512],
                            aoT[:, k, m * 128 : (m + 1) * 128],
                            wo_c[c][:, k, i2 * 512 : (i2 + 1) * 512],
                            start=(k == 0), stop=(k == HQ - 1))
                ost = ostage.tile([128, 1024], f32, tag="ostg", name=f"ost_{m}_{c}")
                if m == IT - 1 and c == 3:
                    # final chunk: evict/DMA in halves on both engines so the
                    # end-of-kernel drain waits on a shorter chain
                    nc.scalar.activation(out=ost[:, 0:512], in_=ps[:, 0:512],
                                         func=AF.Copy)
                    nc.vector.tensor_copy(out=ost[:, 512:1024],
                                          in_=ps[:, 512:1024])
                    for half in range(2):
                        nc.sync.dma_start(
                            out=out.ap()[m * 128 : (m + 1) * 128,
                                         c * 1024 + half * 512
                                         : c * 1024 + (half + 1) * 512],
                            in_=ost[:, half * 512 : (half + 1) * 512])
                    return
                if o_ec[0] % 2 == 0:
                    nc.scalar.activation(out=ost, in_=ps, func=AF.Copy)
                else:
                    nc.vector.tensor_copy(out=ost, in_=ps)
                o_ec[0] += 1
                nc.sync.dma_start(
                    out=out.ap()[m * 128 : (m + 1) * 128,
                                 c * 1024 : (c + 1) * 1024],
                    in_=ost)

            fillers = deque()  # (stamp, fn): PE work lagging the scores

            def flush_one():
                if fillers:
                    fillers.popleft()[1]()
                    return True
                return False

            for ib in range(IB):
                # P3 chunks for the previous block, woven between score pairs
                p3q = deque((m, c) for m in range(ib * 4 - 4, ib * 4)
                            for c in range(4)) if ib > 0 else deque()
                reserve = []
                if ib == IB - 1:
                    # hold back two chunks to cover the last head's exp tail
                    reserve = [p3q.pop(), p3q.pop()]
                for h in range(HQ):
                    gh = ib * HQ + h
                    # All consumers of step gh-2's pt tiles must be emitted
                    # before step gh's exp rewrites those slots (bufs=2).
                    while fillers and fillers[0][0] <= gh - 2:
                        flush_one()
                    pts = {}
                    # diagonal pairs first: their exp+mask chain then has a
                    # full head-step of slack before PV consumes them
                    order = [2 * ib, 2 * ib + 1] + list(range(2 * ib))
                    for p in order:
                        pts.update(emit_score_pair(h, ib, p))
                        if not flush_one() and p3q:
                            m, c = p3q.popleft()
                            emit_p3_chunk(m, c)
                    ao_blk = aopool.tile([128, 512], bf16, tag="aob",
                                         name=f"aob_{ib}_{h}")
                    if h == HQ - 1:
                        for m, c in reserve:
                            fillers.append((gh,
                                lambda m=m, c=c: emit_p3_chunk(m, c)))
                    for itl in range(4):
                        fillers.append((gh,
                            lambda h=h, itl=itl, pts=pts, ao_blk=ao_blk, ib=ib:
                            emit_pv_unit(h, ib, itl, pts, ao_blk)))
                    if p3q:
                        fillers.append((gh,
                            lambda p3q=p3q: emit_p3_chunk(*p3q.popleft())
                            if p3q else None))
                    cell = []
                    fillers.append((gh,
                        lambda h=h, ao_blk=ao_blk, cell=cell, ib=ib:
                        emit_ao_transpose_a(h, ib, ao_blk, cell)))
                    fillers.append((gh,
                        lambda h=h, ao_blk=ao_blk, cell=cell, ib=ib:
                        emit_ao_transpose_b(h, ib, ao_blk, cell)))
                    if ib == 0 and h == 3:
                        # it=14/15's q transposes: pure PE work to fill the
                        # ACT-bound early block, after their RoPE chains drain
                        for pi in range(2):
                            for gi in range(2):
                                fillers.append((gh,
                                    lambda pi=pi, gi=gi: emit_deferred_q(pi, gi)))
                # leftover P3 chunks weave into the next block
                while p3q:
                    m, c = p3q.popleft()
                    fillers.append((ib * HQ + HQ - 1,
                        lambda m=m, c=c: emit_p3_chunk(m, c)))
            while fillers:
                flush_one()

            # Final P3 block (no attention left to hide behind)
            for m in range(12, 16):
                for c in range(4):
                    emit_p3_chunk(m, c)

    return nc


def prep_core_inputs(hidden_states, position_ids, Wq, Wk, Wv, Wo, q_norm_w, k_norm_w):
    """Host-side shard + layout prep. Returns list of 8 in_maps."""
    pos = np.asarray(position_ids).reshape(-1).astype(np.float64)  # [S]
    inv_freq = 1.0 / (
        ROPE_THETA ** (np.arange(0, HEAD_DIM, 2, dtype=np.float64) / HEAD_DIM)
    )  # [64]
    ang = pos[:, None] * inv_freq[None, :]  # [S, 64]
    emb = np.concatenate([ang, ang], axis=1)  # [S, 128]
    scale = HEAD_DIM ** (-0.25)
    cos = (np.cos(emb) * scale).astype(np.float32)  # [S, 128]
    sin = (np.sin(emb) * scale).astype(np.float32)
    qw = np.asarray(q_norm_w, dtype=np.float32)
    kw = np.asarray(k_norm_w, dtype=np.float32)
    qw_roll = np.concatenate([qw[64:], qw[:64]])
    kw_roll = np.concatenate([kw[64:], kw[:64]])

    def table(t):  # [S,128] -> [128, IT, 128]
        return np.ascontiguousarray(
            t.reshape(IT, 128, 128).transpose(1, 0, 2)
        )

    cosq_t = table(cos * qw[None, :]).astype(BF16)
    sinq_t = table(sin * qw_roll[None, :]).astype(BF16)
    cosk_t = table(cos * kw[None, :]).astype(BF16)
    sink_t = table(sin * kw_roll[None, :]).astype(BF16)

    # triangular mask for the exact-diagonal tiles: valid when j <= i
    jj = np.arange(128)[:, None]
    ii = np.arange(128)[None, :]
    trim = (jj <= ii).astype(np.float32).astype(BF16)
    ident = np.eye(128, dtype=np.float32).astype(BF16)

    hs = np.asarray(hidden_states, dtype=np.float32)
    Wq = np.asarray(Wq, dtype=np.float32)
    Wk = np.asarray(Wk, dtype=np.float32)
    Wv = np.asarray(Wv, dtype=np.float32)
    Wo = np.asarray(Wo, dtype=np.float32)

    hst_b = []
    for b in range(B):
        hsT = hs[b].T.astype(BF16)  # [4096, 2048]
        # -> [IT, 128(hid), KT, 128(tok)]
        t = hsT.reshape(KT, 128, IT, 128).transpose(2, 1, 0, 3)
        hst_b.append(np.ascontiguousarray(t))

    in_maps = []
    for c in range(N_CORES):
        b, grp = divmod(c, TP)
        wq_s = Wq[:, grp * HQ * 128 : (grp + 1) * HQ * 128].astype(BF16)
        wq_t = np.ascontiguousarray(
            wq_s.reshape(KT, 128, HQ * 128).transpose(1, 0, 2)
        )  # [128, KT, 1024]
        wk_s = Wk[:, grp * HKV * 128 : (grp + 1) * HKV * 128]
        wv_s = Wv[:, grp * HKV * 128 : (grp + 1) * HKV * 128]
        wkv_s = np.concatenate([wk_s, wv_s], axis=1).astype(BF16)  # [4096, 512]
        wkv_t = np.ascontiguousarray(
            wkv_s.reshape(KT, 128, 512).transpose(1, 0, 2)
        )  # [128, KT, 512]
        wo_s = Wo[grp * HQ * 128 : (grp + 1) * HQ * 128, :].astype(BF16)  # [1024, 4096]
        wo_t = np.ascontiguousarray(
            wo_s.reshape(HQ, 128, HIDDEN).transpose(1, 0, 2)
        )  # [128, HQ, 4096]
        in_maps.append(
            {
                "hst": hst_b[b],
                "wq": wq_t,
                "wkv": wkv_t,
                "wo": wo_t,
                "cosq": cosq_t,
                "sinq": sinq_t,
                "cosk": cosk_t,
                "sink": sink_t,
                "trim": trim,
                "ident": ident,
            }
        )
    return in_maps


def kernel(hidden_states, position_ids, Wq, Wk, Wv, Wo, q_norm_w, k_norm_w,
           _trace=False, _tmpdir=None):
    from concourse.bass_utils import run_bass_kernel_spmd

    nc = build_bass()
    in_maps = prep_core_inputs(
        hidden_states, position_ids, Wq, Wk, Wv, Wo, q_norm_w, k_norm_w
    )
    kwargs = {}
    if _trace:
        kwargs = dict(trace=True, tmpdir=_tmpdir)
    res = run_bass_kernel_spmd(nc, in_maps, list(range(N_CORES)), **kwargs)
    partials = [res.results[c]["out"] for c in range(N_CORES)]
    outb = [
        np.sum([partials[b * TP + g] for g in range(TP)], axis=0, dtype=np.float32)
        for b in range(B)
    ]
    full = np.stack(outb).astype(np.float32)  # [2, 2048, 4096]
    if _trace:
        kernel._last_result = res
    return full
